# revision 14
# baseline (speedup 1.0000x reference)
"""BoxHead MLP on 8 Trainium2 NeuronCores.

Computes, for x[20000, 12544]:
    h1 = relu(x @ W1 + b1)          # [N, 1024]
    h2 = relu(h1 @ W2 + b2)         # [N, 1024]
    class_logits = h2 @ Wc + bc     # [N, 4]
    box_pred     = h2 @ Wr + br     # [N, 12]

Strategy: data-parallel over the proposal dim (2500 rows/core, padded to
2560 = 5 chunks of 512), weights replicated.  All activations are kept
feature-major ([hidden, proposals]) so the kernel needs no on-device
transposes: the host pre-transposes x into [chunk, part, ktile, col]
layout.  Matmuls run in bf16 with fp32 PSUM accumulation; biases + ReLU
are applied by ScalarE while draining PSUM.
"""

import os
import sys

if "/opt/trn_rl_repo" not in sys.path:
    sys.path.insert(0, "/opt/trn_rl_repo")

import numpy as np
import ml_dtypes

BF16 = ml_dtypes.bfloat16

N_CORES = 8
N_FULL = 20000
ROWS = N_FULL // N_CORES      # 2500 proposals per core
NT = 512                      # max chunk width (one PSUM bank of fp32)
CHUNKS = [512, 512, 512, 512, 452]   # exactly 2500 columns
CH = len(CHUNKS)
ROWS_PAD = CH * NT            # 2560 (x DRAM layout is padded; compute is not)
D_IN = 12544
H = 1024
KT1 = D_IN // 128             # 98 k-tiles for layer 1
KG = 14                       # k-tiles per x/W1 stream group
NG = KT1 // KG                # 7 groups
MO = H // 128                 # 8 hidden tiles
NH = 16                       # 4 class + 12 box outputs

_CACHE = {}


def _patch_tile_drain():
    """This walrus build accepts only ONE sync-wait per instruction, but
    TileContext's tail drain accumulates one wait per DMA lane.  Split the
    waits: one single-wait NOP per logical proc, then a bare drain."""
    import concourse.tile as tile
    from concourse.vector_clock import ScopedClock

    if getattr(tile.TileContext, "_drain_split_patched", False):
        return

    def _split_drain_and_barrier(self, tick_clock, wait_clock):
        nc = self.nc
        vclock = tick_clock.global_clock
        for proc in range(len(vclock)):
            t = vclock[proc]
            if t <= 0:
                continue
            sub = ScopedClock()
            sub.require_at_least(None, proc, t)
            nop = nc.sync.nop(nofuse=True, hint=f"drain_wait_p{proc}")
            wait_clock.add_sem_waits(nop.ins, sub)
        nc.sync.drain()
        nc.all_engine_barrier()
        assert self.sems is not None
        popped = nc._tile_sem_poison_stack.pop()
        assert popped is self._sem_poison
        nc.clear_and_free_semaphores(list(self.sems.allocated().values()))
        nc.all_engine_barrier()

    tile.TileContext._drain_and_barrier = _split_drain_and_barrier
    tile.TileContext._drain_split_patched = True


def _split_multi_wait_insts(bir_json):
    """This walrus build accepts only ONE sync-wait per instruction; Tile
    emits instructions with several.  Rewrite the BIR: move all but the
    last wait of each instruction onto fresh single-wait NoOps inserted
    just before it on the same engine stream (identical semantics — the
    sequencer blocks at each wait in order)."""
    import json as _json

    bir = _json.loads(bir_json)
    ctr = 0
    for fn in bir.get("functions", []):
        for bb in fn.get("blocks", []):
            out = []
            for inst in bb.get("instructions", []):
                si = inst.get("sync_info")
                waits = (si or {}).get("on_wait") or []
                if len(waits) > 1:
                    for w in waits[:-1]:
                        ctr += 1
                        out.append({
                            "debug": inst.get("debug", 0),
                            "engine": inst["engine"],
                            "ins": [],
                            "name": f"I-wsplit-{ctr}",
                            "opcode": "NoOp",
                            "outs": [],
                            "sync_info": {"on_update": [], "on_wait": [w]},
                            "text_hint": "wsplit",
                        })
                    si["on_wait"] = [waits[-1]]
                out.append(inst)
            bb["instructions"] = out
    return _json.dumps(bir).encode()


def _patch_compile_for_single_wait_walrus():
    import concourse.bass_utils as bass_utils
    import concourse.bass2jax as bass2jax

    if getattr(bass_utils, "_wsplit_patched", False):
        return
    orig = bass_utils.compile_bir_kernel

    def patched(bir_json, tmpdir, neff_name="file.neff"):
        return orig(_split_multi_wait_insts(bir_json), tmpdir, neff_name)

    bass_utils.compile_bir_kernel = patched
    bass2jax.compile_bir_kernel = patched
    bass_utils._wsplit_patched = True


def _build_nc():
    import concourse.bass as bass
    import concourse.mybir as mybir
    import concourse.tile as tile

    _patch_tile_drain()
    _patch_compile_for_single_wait_walrus()
    dt = mybir.dt
    Relu = mybir.ActivationFunctionType.Relu

    nc = bass.Bass()
    x_in = nc.dram_tensor("x_in", [CH, 128, KT1, NT], dt.bfloat16, kind="ExternalInput")
    w1_in = nc.dram_tensor("w1_in", [128, KT1, H], dt.bfloat16, kind="ExternalInput")
    w2_in = nc.dram_tensor("w2_in", [128, MO, H], dt.bfloat16, kind="ExternalInput")
    wh_in = nc.dram_tensor("wh_in", [128, MO, NH], dt.bfloat16, kind="ExternalInput")
    b1_in = nc.dram_tensor("b1_in", [128, MO], dt.float32, kind="ExternalInput")
    b2_in = nc.dram_tensor("b2_in", [128, MO], dt.float32, kind="ExternalInput")
    bh_in = nc.dram_tensor("bh_in", [NH, 1], dt.float32, kind="ExternalInput")
    out_t = nc.dram_tensor("out_t", [NH, ROWS], dt.float32, kind="ExternalOutput")

    with tile.TileContext(nc) as tc:
        with (
            tc.tile_pool(name="singles", bufs=1) as singles,
            tc.tile_pool(name="xp", bufs=2) as xpool,
            tc.tile_pool(name="wp", bufs=3) as wpool,
            tc.tile_pool(name="h1p", bufs=2) as h1pool,
            tc.tile_pool(name="h2p", bufs=2) as h2pool,
            tc.tile_pool(name="ps", bufs=8, space="PSUM") as pspool,
        ):
            # Resident weights/biases/outputs (loaded after the first x/W1
            # stream groups are issued; nothing needs them until L1 drains).
            w2_sb = singles.tile([128, MO, H], dt.bfloat16)
            wh_sb = singles.tile([128, MO, NH], dt.bfloat16)
            b1_sb = singles.tile([128, MO], dt.float32)
            b2_sb = singles.tile([128, MO], dt.float32)
            bh_sb = singles.tile([NH, 1], dt.float32)
            out_sb = singles.tile([NH, ROWS], dt.float32)

            for c, nt in enumerate(CHUNKS):
                n0 = c * NT
                ps1 = [pspool.tile([128, NT], dt.float32, tag="mm",
                                   name=f"ps1_{c}_{m}") for m in range(MO)]
                h1 = h1pool.tile([128, MO, NT], dt.bfloat16, tag="h1",
                                 name=f"h1_{c}")
                for g in range(NG):
                    k0 = g * KG
                    # Startup is DMA-bandwidth-bound: feed the first groups
                    # in escalating pieces so the PE starts after ~0.4 MB
                    # and never starves while the rest streams in.
                    if c == 0 and g == 0:
                        pieces = [(0, 1), (1, 2), (2, 4), (4, 8), (8, KG)]
                    elif c == 0 and g == 1:
                        pieces = [(0, 7), (7, KG)]
                    else:
                        pieces = [(0, KG)]
                    xg = xpool.tile([128, KG, NT], dt.bfloat16, tag="x",
                                    name=f"x_{c}_{g}")
                    wg = wpool.tile([128, KG, H], dt.bfloat16, tag="w1",
                                    name=f"w_{c}_{g}")
                    for (pa, pb) in pieces:
                        nc.sync.dma_start(out=xg[:, pa:pb, :nt],
                                          in_=x_in[c, :, k0 + pa:k0 + pb, :nt])
                        nc.sync.dma_start(out=wg[:, pa:pb, :],
                                          in_=w1_in[:, k0 + pa:k0 + pb, :])
                    kparts = [(xg, kk, wg, kk) for kk in range(KG)]
                    if c == 0 and g == 1:
                        nc.sync.dma_start(out=w2_sb, in_=w2_in[:, :, :])
                        nc.sync.dma_start(out=wh_sb, in_=wh_in[:, :, :])
                        nc.sync.dma_start(out=b1_sb, in_=b1_in[:, :])
                        nc.sync.dma_start(out=b2_sb, in_=b2_in[:, :])
                        nc.sync.dma_start(out=bh_sb, in_=bh_in[:, :])
                    if g < NG - 1:
                        for kk in range(KG):
                            k = k0 + kk
                            xt, xi, wt, wi = kparts[kk]
                            for m in range(MO):
                                nc.tensor.matmul(
                                    ps1[m][:, :nt],
                                    lhsT=wt[:, wi, m * 128:(m + 1) * 128],
                                    rhs=xt[:, xi, :nt],
                                    start=(k == 0),
                                    stop=False,
                                )
                    else:
                        # Last k-group m-major: each m-tile finishes 14 MMs
                        # apart, so its ReLU (and PSUM bank free) pipelines
                        # behind the PE instead of serializing at the end.
                        for m in range(MO):
                            for kk in range(KG):
                                xt, xi, wt, wi = kparts[kk]
                                nc.tensor.matmul(
                                    ps1[m][:, :nt],
                                    lhsT=wt[:, wi, m * 128:(m + 1) * 128],
                                    rhs=xt[:, xi, :nt],
                                    start=False,
                                    stop=(kk == KG - 1),
                                )
                            nc.scalar.activation(
                                h1[:, m, :nt], ps1[m][:, :nt], Relu,
                                bias=b1_sb[:, m:m + 1], scale=1.0,
                            )

                # L2 m2-outer / k2-inner: h1 is resident, so only ~2 PSUM
                # banks stay live and banks free early for the next chunk.
                h2 = h2pool.tile([128, MO, NT], dt.bfloat16, tag="h2",
                                 name=f"h2_{c}")
                for m2 in range(MO):
                    ps2 = pspool.tile([128, NT], dt.float32, tag="mm",
                                      name=f"ps2_{c}_{m2}")
                    for k2 in range(MO):
                        nc.tensor.matmul(
                            ps2[:, :nt],
                            lhsT=w2_sb[:, k2, m2 * 128:(m2 + 1) * 128],
                            rhs=h1[:, k2, :nt],
                            start=(k2 == 0),
                            stop=(k2 == MO - 1),
                        )
                    nc.scalar.activation(
                        h2[:, m2, :nt], ps2[:, :nt], Relu,
                        bias=b2_sb[:, m2:m2 + 1], scale=1.0,
                    )

                psh = pspool.tile([NH, NT], dt.float32, tag="mm",
                                  name=f"psh_{c}")
                for k2 in range(MO):
                    nc.tensor.matmul(
                        psh[:, :nt],
                        lhsT=wh_sb[:, k2, :],
                        rhs=h2[:, k2, :nt],
                        start=(k2 == 0),
                        stop=(k2 == MO - 1),
                    )
                nc.vector.tensor_scalar_add(
                    out=out_sb[:, n0:n0 + nt], in0=psh[:, :nt], scalar1=bh_sb
                )
                nc.sync.dma_start(out=out_t[:, n0:n0 + nt],
                                  in_=out_sb[:, n0:n0 + nt])

    return nc


def _prep_core_x(x_shard_f32):
    """[2500, 12544] f32 -> [CH, 128, KT1, NT] bf16 with
    out[c, p, ko, n] = x[c*NT + n, ko*128 + p] (rows padded with zeros)."""
    xp = np.zeros((ROWS_PAD, D_IN), dtype=BF16)
    xp[:ROWS] = x_shard_f32.astype(BF16)
    v = xp.reshape(CH, NT, KT1, 128)
    return np.ascontiguousarray(np.transpose(v, (0, 3, 2, 1)))


def kernel(x, W1, b1, W2, b2, Wc, bc, Wr, br):
    from concourse.bass_utils import run_bass_kernel_spmd

    x = np.asarray(x, dtype=np.float32)
    W1 = np.asarray(W1, dtype=np.float32)
    W2 = np.asarray(W2, dtype=np.float32)
    Wc = np.asarray(Wc, dtype=np.float32)
    Wr = np.asarray(Wr, dtype=np.float32)
    b1 = np.asarray(b1, dtype=np.float32)
    b2 = np.asarray(b2, dtype=np.float32)
    bc = np.asarray(bc, dtype=np.float32)
    br = np.asarray(br, dtype=np.float32)

    # Weight layouts: [p, ktile, free] with contraction index = ktile*128 + p.
    w1_dev = np.ascontiguousarray(
        W1.astype(BF16).reshape(KT1, 128, H).transpose(1, 0, 2)
    )
    w2_dev = np.ascontiguousarray(
        W2.astype(BF16).reshape(MO, 128, H).transpose(1, 0, 2)
    )
    wh = np.concatenate([Wc, Wr], axis=1)  # [H, 16]
    wh_dev = np.ascontiguousarray(
        wh.astype(BF16).reshape(MO, 128, NH).transpose(1, 0, 2)
    )
    b1_dev = np.ascontiguousarray(b1.reshape(MO, 128).T)
    b2_dev = np.ascontiguousarray(b2.reshape(MO, 128).T)
    bh_dev = np.ascontiguousarray(
        np.concatenate([bc, br]).reshape(NH, 1).astype(np.float32)
    )

    in_maps = []
    for c in range(N_CORES):
        x_dev = _prep_core_x(x[c * ROWS:(c + 1) * ROWS])
        in_maps.append({
            "x_in": x_dev,
            "w1_in": w1_dev,
            "w2_in": w2_dev,
            "wh_in": wh_dev,
            "b1_in": b1_dev,
            "b2_in": b2_dev,
            "bh_in": bh_dev,
        })

    if "nc" not in _CACHE:
        _CACHE["nc"] = _build_nc()
    nc = _CACHE["nc"]

    res = run_bass_kernel_spmd(nc, in_maps, core_ids=list(range(N_CORES)))
    kernel.last_results = res

    outs = []
    for c in range(N_CORES):
        o = res.results[c]["out_t"]          # [16, 2500] f32
        outs.append(o.T)                     # [2500, 16]
    full = np.concatenate(outs, axis=0)      # [20000, 16]
    class_logits = np.ascontiguousarray(full[:, :4])
    box_pred = np.ascontiguousarray(full[:, 4:])
    return class_logits, box_pred


# revision 15
# speedup vs baseline: 1.0063x; 1.0063x over previous
"""BoxHead MLP on 8 Trainium2 NeuronCores.

Computes, for x[20000, 12544]:
    h1 = relu(x @ W1 + b1)          # [N, 1024]
    h2 = relu(h1 @ W2 + b2)         # [N, 1024]
    class_logits = h2 @ Wc + bc     # [N, 4]
    box_pred     = h2 @ Wr + br     # [N, 12]

Strategy: data-parallel over the proposal dim (2500 rows/core, padded to
2560 = 5 chunks of 512), weights replicated.  All activations are kept
feature-major ([hidden, proposals]) so the kernel needs no on-device
transposes: the host pre-transposes x into [chunk, part, ktile, col]
layout.  Matmuls run in bf16 with fp32 PSUM accumulation; biases + ReLU
are applied by ScalarE while draining PSUM.
"""

import os
import sys

if "/opt/trn_rl_repo" not in sys.path:
    sys.path.insert(0, "/opt/trn_rl_repo")

import numpy as np
import ml_dtypes

BF16 = ml_dtypes.bfloat16

N_CORES = 8
N_FULL = 20000
ROWS = N_FULL // N_CORES      # 2500 proposals per core
NT = 512                      # max chunk width (one PSUM bank of fp32)
CHUNKS = [512, 512, 512, 512, 452]   # exactly 2500 columns
CH = len(CHUNKS)
ROWS_PAD = CH * NT            # 2560 (x DRAM layout is padded; compute is not)
D_IN = 12544
H = 1024
KT1 = D_IN // 128             # 98 k-tiles for layer 1
KG = 14                       # k-tiles per x/W1 stream group
NG = KT1 // KG                # 7 groups
MO = H // 128                 # 8 hidden tiles
NH = 16                       # 4 class + 12 box outputs

_CACHE = {}


def _patch_tile_drain():
    """This walrus build accepts only ONE sync-wait per instruction, but
    TileContext's tail drain accumulates one wait per DMA lane.  Split the
    waits: one single-wait NOP per logical proc, then a bare drain."""
    import concourse.tile as tile
    from concourse.vector_clock import ScopedClock

    if getattr(tile.TileContext, "_drain_split_patched", False):
        return

    def _split_drain_and_barrier(self, tick_clock, wait_clock):
        nc = self.nc
        vclock = tick_clock.global_clock
        for proc in range(len(vclock)):
            t = vclock[proc]
            if t <= 0:
                continue
            sub = ScopedClock()
            sub.require_at_least(None, proc, t)
            nop = nc.sync.nop(nofuse=True, hint=f"drain_wait_p{proc}")
            wait_clock.add_sem_waits(nop.ins, sub)
        nc.sync.drain()
        nc.all_engine_barrier()
        assert self.sems is not None
        popped = nc._tile_sem_poison_stack.pop()
        assert popped is self._sem_poison
        nc.clear_and_free_semaphores(list(self.sems.allocated().values()))
        nc.all_engine_barrier()

    tile.TileContext._drain_and_barrier = _split_drain_and_barrier
    tile.TileContext._drain_split_patched = True


def _split_multi_wait_insts(bir_json):
    """This walrus build accepts only ONE sync-wait per instruction; Tile
    emits instructions with several.  Rewrite the BIR: move all but the
    last wait of each instruction onto fresh single-wait NoOps inserted
    just before it on the same engine stream (identical semantics — the
    sequencer blocks at each wait in order)."""
    import json as _json

    bir = _json.loads(bir_json)
    ctr = 0
    for fn in bir.get("functions", []):
        for bb in fn.get("blocks", []):
            out = []
            for inst in bb.get("instructions", []):
                si = inst.get("sync_info")
                waits = (si or {}).get("on_wait") or []
                if len(waits) > 1:
                    for w in waits[:-1]:
                        ctr += 1
                        out.append({
                            "debug": inst.get("debug", 0),
                            "engine": inst["engine"],
                            "ins": [],
                            "name": f"I-wsplit-{ctr}",
                            "opcode": "NoOp",
                            "outs": [],
                            "sync_info": {"on_update": [], "on_wait": [w]},
                            "text_hint": "wsplit",
                        })
                    si["on_wait"] = [waits[-1]]
                out.append(inst)
            bb["instructions"] = out
    return _json.dumps(bir).encode()


def _patch_compile_for_single_wait_walrus():
    import concourse.bass_utils as bass_utils
    import concourse.bass2jax as bass2jax

    if getattr(bass_utils, "_wsplit_patched", False):
        return
    orig = bass_utils.compile_bir_kernel

    def patched(bir_json, tmpdir, neff_name="file.neff"):
        return orig(_split_multi_wait_insts(bir_json), tmpdir, neff_name)

    bass_utils.compile_bir_kernel = patched
    bass2jax.compile_bir_kernel = patched
    bass_utils._wsplit_patched = True


def _build_nc():
    import concourse.bass as bass
    import concourse.mybir as mybir
    import concourse.tile as tile

    _patch_tile_drain()
    _patch_compile_for_single_wait_walrus()
    dt = mybir.dt
    Relu = mybir.ActivationFunctionType.Relu

    nc = bass.Bass()
    x_in = nc.dram_tensor("x_in", [CH, 128, KT1, NT], dt.bfloat16, kind="ExternalInput")
    w1_in = nc.dram_tensor("w1_in", [128, KT1, H], dt.bfloat16, kind="ExternalInput")
    w2_in = nc.dram_tensor("w2_in", [128, MO, H], dt.bfloat16, kind="ExternalInput")
    wh_in = nc.dram_tensor("wh_in", [128, MO, NH], dt.bfloat16, kind="ExternalInput")
    b1_in = nc.dram_tensor("b1_in", [128, MO], dt.float32, kind="ExternalInput")
    b2_in = nc.dram_tensor("b2_in", [128, MO], dt.float32, kind="ExternalInput")
    bh_in = nc.dram_tensor("bh_in", [NH, 1], dt.float32, kind="ExternalInput")
    out_t = nc.dram_tensor("out_t", [NH, ROWS], dt.float32, kind="ExternalOutput")

    with tile.TileContext(nc) as tc:
        with (
            tc.tile_pool(name="singles", bufs=1) as singles,
            tc.tile_pool(name="xp", bufs=3) as xpool,
            tc.tile_pool(name="wp", bufs=2) as wpool,
            tc.tile_pool(name="h1p", bufs=2) as h1pool,
            tc.tile_pool(name="h2p", bufs=2) as h2pool,
            tc.tile_pool(name="ps", bufs=8, space="PSUM") as pspool,
        ):
            # Resident weights/biases/outputs (loaded after the first x/W1
            # stream groups are issued; nothing needs them until L1 drains).
            w2_sb = singles.tile([128, MO, H], dt.bfloat16)
            wh_sb = singles.tile([128, MO, NH], dt.bfloat16)
            b1_sb = singles.tile([128, MO], dt.float32)
            b2_sb = singles.tile([128, MO], dt.float32)
            bh_sb = singles.tile([NH, 1], dt.float32)
            out_sb = singles.tile([NH, ROWS], dt.float32)

            for c, nt in enumerate(CHUNKS):
                n0 = c * NT
                ps1 = [pspool.tile([128, NT], dt.float32, tag="mm",
                                   name=f"ps1_{c}_{m}") for m in range(MO)]
                h1 = h1pool.tile([128, MO, NT], dt.bfloat16, tag="h1",
                                 name=f"h1_{c}")
                for g in range(NG):
                    k0 = g * KG
                    # Startup is DMA-bandwidth-bound: feed the first groups
                    # in escalating pieces so the PE starts after ~0.4 MB
                    # and never starves while the rest streams in.
                    if c == 0 and g == 0:
                        pieces = [(0, 1), (1, 2), (2, 4), (4, 8), (8, KG)]
                    elif c == 0 and g == 1:
                        pieces = [(0, 7), (7, KG)]
                    else:
                        pieces = [(0, KG)]
                    xg = xpool.tile([128, KG, NT], dt.bfloat16, tag="x",
                                    name=f"x_{c}_{g}")
                    wg = wpool.tile([128, KG, H], dt.bfloat16, tag="w1",
                                    name=f"w_{c}_{g}")
                    for (pa, pb) in pieces:
                        nc.sync.dma_start(out=xg[:, pa:pb, :nt],
                                          in_=x_in[c, :, k0 + pa:k0 + pb, :nt])
                        nc.sync.dma_start(out=wg[:, pa:pb, :],
                                          in_=w1_in[:, k0 + pa:k0 + pb, :])
                    kparts = [(xg, kk, wg, kk) for kk in range(KG)]
                    if c == 0 and g == 1:
                        nc.sync.dma_start(out=w2_sb, in_=w2_in[:, :, :])
                        nc.sync.dma_start(out=wh_sb, in_=wh_in[:, :, :])
                        nc.sync.dma_start(out=b1_sb, in_=b1_in[:, :])
                        nc.sync.dma_start(out=b2_sb, in_=b2_in[:, :])
                        nc.sync.dma_start(out=bh_sb, in_=bh_in[:, :])
                    if g < NG - 1:
                        for kk in range(KG):
                            k = k0 + kk
                            xt, xi, wt, wi = kparts[kk]
                            for m in range(MO):
                                nc.tensor.matmul(
                                    ps1[m][:, :nt],
                                    lhsT=wt[:, wi, m * 128:(m + 1) * 128],
                                    rhs=xt[:, xi, :nt],
                                    start=(k == 0),
                                    stop=False,
                                )
                    else:
                        # Last k-group m-major: each m-tile finishes 14 MMs
                        # apart, so its ReLU (and PSUM bank free) pipelines
                        # behind the PE instead of serializing at the end.
                        for m in range(MO):
                            for kk in range(KG):
                                xt, xi, wt, wi = kparts[kk]
                                nc.tensor.matmul(
                                    ps1[m][:, :nt],
                                    lhsT=wt[:, wi, m * 128:(m + 1) * 128],
                                    rhs=xt[:, xi, :nt],
                                    start=False,
                                    stop=(kk == KG - 1),
                                )
                            nc.scalar.activation(
                                h1[:, m, :nt], ps1[m][:, :nt], Relu,
                                bias=b1_sb[:, m:m + 1], scale=1.0,
                            )

                # L2 m2-outer / k2-inner: h1 is resident, so only ~2 PSUM
                # banks stay live and banks free early for the next chunk.
                h2 = h2pool.tile([128, MO, NT], dt.bfloat16, tag="h2",
                                 name=f"h2_{c}")
                for m2 in range(MO):
                    ps2 = pspool.tile([128, NT], dt.float32, tag="mm",
                                      name=f"ps2_{c}_{m2}")
                    for k2 in range(MO):
                        nc.tensor.matmul(
                            ps2[:, :nt],
                            lhsT=w2_sb[:, k2, m2 * 128:(m2 + 1) * 128],
                            rhs=h1[:, k2, :nt],
                            start=(k2 == 0),
                            stop=(k2 == MO - 1),
                        )
                    nc.scalar.activation(
                        h2[:, m2, :nt], ps2[:, :nt], Relu,
                        bias=b2_sb[:, m2:m2 + 1], scale=1.0,
                    )

                psh = pspool.tile([NH, NT], dt.float32, tag="mm",
                                  name=f"psh_{c}")
                for k2 in range(MO):
                    nc.tensor.matmul(
                        psh[:, :nt],
                        lhsT=wh_sb[:, k2, :],
                        rhs=h2[:, k2, :nt],
                        start=(k2 == 0),
                        stop=(k2 == MO - 1),
                    )
                nc.vector.tensor_scalar_add(
                    out=out_sb[:, n0:n0 + nt], in0=psh[:, :nt], scalar1=bh_sb
                )
                nc.sync.dma_start(out=out_t[:, n0:n0 + nt],
                                  in_=out_sb[:, n0:n0 + nt])

    return nc


def _prep_core_x(x_shard_f32):
    """[2500, 12544] f32 -> [CH, 128, KT1, NT] bf16 with
    out[c, p, ko, n] = x[c*NT + n, ko*128 + p] (rows padded with zeros)."""
    xp = np.zeros((ROWS_PAD, D_IN), dtype=BF16)
    xp[:ROWS] = x_shard_f32.astype(BF16)
    v = xp.reshape(CH, NT, KT1, 128)
    return np.ascontiguousarray(np.transpose(v, (0, 3, 2, 1)))


def kernel(x, W1, b1, W2, b2, Wc, bc, Wr, br):
    from concourse.bass_utils import run_bass_kernel_spmd

    x = np.asarray(x, dtype=np.float32)
    W1 = np.asarray(W1, dtype=np.float32)
    W2 = np.asarray(W2, dtype=np.float32)
    Wc = np.asarray(Wc, dtype=np.float32)
    Wr = np.asarray(Wr, dtype=np.float32)
    b1 = np.asarray(b1, dtype=np.float32)
    b2 = np.asarray(b2, dtype=np.float32)
    bc = np.asarray(bc, dtype=np.float32)
    br = np.asarray(br, dtype=np.float32)

    # Weight layouts: [p, ktile, free] with contraction index = ktile*128 + p.
    w1_dev = np.ascontiguousarray(
        W1.astype(BF16).reshape(KT1, 128, H).transpose(1, 0, 2)
    )
    w2_dev = np.ascontiguousarray(
        W2.astype(BF16).reshape(MO, 128, H).transpose(1, 0, 2)
    )
    wh = np.concatenate([Wc, Wr], axis=1)  # [H, 16]
    wh_dev = np.ascontiguousarray(
        wh.astype(BF16).reshape(MO, 128, NH).transpose(1, 0, 2)
    )
    b1_dev = np.ascontiguousarray(b1.reshape(MO, 128).T)
    b2_dev = np.ascontiguousarray(b2.reshape(MO, 128).T)
    bh_dev = np.ascontiguousarray(
        np.concatenate([bc, br]).reshape(NH, 1).astype(np.float32)
    )

    in_maps = []
    for c in range(N_CORES):
        x_dev = _prep_core_x(x[c * ROWS:(c + 1) * ROWS])
        in_maps.append({
            "x_in": x_dev,
            "w1_in": w1_dev,
            "w2_in": w2_dev,
            "wh_in": wh_dev,
            "b1_in": b1_dev,
            "b2_in": b2_dev,
            "bh_in": bh_dev,
        })

    if "nc" not in _CACHE:
        _CACHE["nc"] = _build_nc()
    nc = _CACHE["nc"]

    res = run_bass_kernel_spmd(nc, in_maps, core_ids=list(range(N_CORES)))
    kernel.last_results = res

    outs = []
    for c in range(N_CORES):
        o = res.results[c]["out_t"]          # [16, 2500] f32
        outs.append(o.T)                     # [2500, 16]
    full = np.concatenate(outs, axis=0)      # [20000, 16]
    class_logits = np.ascontiguousarray(full[:, :4])
    box_pred = np.ascontiguousarray(full[:, 4:])
    return class_logits, box_pred


# revision 16
# speedup vs baseline: 1.0116x; 1.0052x over previous
"""BoxHead MLP on 8 Trainium2 NeuronCores.

Computes, for x[20000, 12544]:
    h1 = relu(x @ W1 + b1)          # [N, 1024]
    h2 = relu(h1 @ W2 + b2)         # [N, 1024]
    class_logits = h2 @ Wc + bc     # [N, 4]
    box_pred     = h2 @ Wr + br     # [N, 12]

Strategy: data-parallel over the proposal dim (2500 rows/core, padded to
2560 = 5 chunks of 512), weights replicated.  All activations are kept
feature-major ([hidden, proposals]) so the kernel needs no on-device
transposes: the host pre-transposes x into [chunk, part, ktile, col]
layout.  Matmuls run in bf16 with fp32 PSUM accumulation; biases + ReLU
are applied by ScalarE while draining PSUM.
"""

import os
import sys

if "/opt/trn_rl_repo" not in sys.path:
    sys.path.insert(0, "/opt/trn_rl_repo")

import numpy as np
import ml_dtypes

BF16 = ml_dtypes.bfloat16

N_CORES = 8
N_FULL = 20000
ROWS = N_FULL // N_CORES      # 2500 proposals per core
NT = 512                      # max chunk width (one PSUM bank of fp32)
CHUNKS = [512, 512, 512, 512, 452]   # exactly 2500 columns
CH = len(CHUNKS)
ROWS_PAD = CH * NT            # 2560 (x DRAM layout is padded; compute is not)
D_IN = 12544
H = 1024
KT1 = D_IN // 128             # 98 k-tiles for layer 1
KG = 14                       # k-tiles per x/W1 stream group
NG = KT1 // KG                # 7 groups
MO = H // 128                 # 8 hidden tiles
NH = 16                       # 4 class + 12 box outputs

_CACHE = {}


def _patch_tile_drain():
    """This walrus build accepts only ONE sync-wait per instruction, but
    TileContext's tail drain accumulates one wait per DMA lane.  Split the
    waits: one single-wait NOP per logical proc, then a bare drain."""
    import concourse.tile as tile
    from concourse.vector_clock import ScopedClock

    if getattr(tile.TileContext, "_drain_split_patched", False):
        return

    def _split_drain_and_barrier(self, tick_clock, wait_clock):
        nc = self.nc
        vclock = tick_clock.global_clock
        for proc in range(len(vclock)):
            t = vclock[proc]
            if t <= 0:
                continue
            sub = ScopedClock()
            sub.require_at_least(None, proc, t)
            nop = nc.sync.nop(nofuse=True, hint=f"drain_wait_p{proc}")
            wait_clock.add_sem_waits(nop.ins, sub)
        nc.sync.drain()
        nc.all_engine_barrier()
        assert self.sems is not None
        popped = nc._tile_sem_poison_stack.pop()
        assert popped is self._sem_poison
        nc.clear_and_free_semaphores(list(self.sems.allocated().values()))
        nc.all_engine_barrier()

    tile.TileContext._drain_and_barrier = _split_drain_and_barrier
    tile.TileContext._drain_split_patched = True


def _split_multi_wait_insts(bir_json):
    """This walrus build accepts only ONE sync-wait per instruction; Tile
    emits instructions with several.  Rewrite the BIR: move all but the
    last wait of each instruction onto fresh single-wait NoOps inserted
    just before it on the same engine stream (identical semantics — the
    sequencer blocks at each wait in order)."""
    import json as _json

    bir = _json.loads(bir_json)
    ctr = 0
    for fn in bir.get("functions", []):
        for bb in fn.get("blocks", []):
            out = []
            for inst in bb.get("instructions", []):
                si = inst.get("sync_info")
                waits = (si or {}).get("on_wait") or []
                if len(waits) > 1:
                    for w in waits[:-1]:
                        ctr += 1
                        out.append({
                            "debug": inst.get("debug", 0),
                            "engine": inst["engine"],
                            "ins": [],
                            "name": f"I-wsplit-{ctr}",
                            "opcode": "NoOp",
                            "outs": [],
                            "sync_info": {"on_update": [], "on_wait": [w]},
                            "text_hint": "wsplit",
                        })
                    si["on_wait"] = [waits[-1]]
                out.append(inst)
            bb["instructions"] = out
    return _json.dumps(bir).encode()


def _patch_compile_for_single_wait_walrus():
    import concourse.bass_utils as bass_utils
    import concourse.bass2jax as bass2jax

    if getattr(bass_utils, "_wsplit_patched", False):
        return
    orig = bass_utils.compile_bir_kernel

    def patched(bir_json, tmpdir, neff_name="file.neff"):
        return orig(_split_multi_wait_insts(bir_json), tmpdir, neff_name)

    bass_utils.compile_bir_kernel = patched
    bass2jax.compile_bir_kernel = patched
    bass_utils._wsplit_patched = True


def _build_nc():
    import concourse.bass as bass
    import concourse.mybir as mybir
    import concourse.tile as tile

    _patch_tile_drain()
    _patch_compile_for_single_wait_walrus()
    dt = mybir.dt
    Relu = mybir.ActivationFunctionType.Relu

    nc = bass.Bass()
    x_in = nc.dram_tensor("x_in", [CH, 128, KT1, NT], dt.bfloat16, kind="ExternalInput")
    w1_in = nc.dram_tensor("w1_in", [128, KT1, H], dt.bfloat16, kind="ExternalInput")
    w2_in = nc.dram_tensor("w2_in", [128, MO, H], dt.bfloat16, kind="ExternalInput")
    wh_in = nc.dram_tensor("wh_in", [128, MO, NH], dt.bfloat16, kind="ExternalInput")
    b1_in = nc.dram_tensor("b1_in", [128, MO], dt.float32, kind="ExternalInput")
    b2_in = nc.dram_tensor("b2_in", [128, MO], dt.float32, kind="ExternalInput")
    bh_in = nc.dram_tensor("bh_in", [NH, 1], dt.float32, kind="ExternalInput")
    out_t = nc.dram_tensor("out_t", [NH, ROWS], dt.float32, kind="ExternalOutput")

    with tile.TileContext(nc) as tc:
        with (
            tc.tile_pool(name="singles", bufs=1) as singles,
            tc.tile_pool(name="xp", bufs=3) as xpool,
            tc.tile_pool(name="wp", bufs=2) as wpool,
            tc.tile_pool(name="h1p", bufs=2) as h1pool,
            tc.tile_pool(name="h2p", bufs=2) as h2pool,
            tc.tile_pool(name="ps", bufs=8, space="PSUM") as pspool,
        ):
            # Resident weights/biases/outputs (loaded after the first x/W1
            # stream groups are issued; nothing needs them until L1 drains).
            w2_sb = singles.tile([128, MO, H], dt.bfloat16)
            wh_sb = singles.tile([128, MO, NH], dt.bfloat16)
            b1_sb = singles.tile([128, MO], dt.float32)
            b2_sb = singles.tile([128, MO], dt.float32)
            bh_sb = singles.tile([NH, 1], dt.float32)
            out_sb = singles.tile([NH, ROWS], dt.float32)

            for c, nt in enumerate(CHUNKS):
                n0 = c * NT
                ps1 = [pspool.tile([128, NT], dt.float32, tag="mm",
                                   name=f"ps1_{c}_{m}") for m in range(MO)]
                h1 = h1pool.tile([128, MO, NT], dt.bfloat16, tag="h1",
                                 name=f"h1_{c}")
                for g in range(NG):
                    k0 = g * KG
                    # Startup is DMA-bandwidth-bound: feed the first groups
                    # in escalating pieces so the PE starts after ~0.4 MB
                    # and never starves while the rest streams in.
                    if c == 0 and g == 0:
                        pieces = [(0, 1), (1, 2), (2, 3), (3, 5), (5, 8),
                                  (8, 11), (11, KG)]
                    elif c == 0 and g in (1, 2):
                        pieces = [(0, 5), (5, 10), (10, KG)]
                    else:
                        pieces = [(0, KG)]
                    xg = xpool.tile([128, KG, NT], dt.bfloat16, tag="x",
                                    name=f"x_{c}_{g}")
                    wg = wpool.tile([128, KG, H], dt.bfloat16, tag="w1",
                                    name=f"w_{c}_{g}")
                    for (pa, pb) in pieces:
                        # W1 slice first: LDWEIGHTS consumes it before the
                        # matmul streams the x slice.
                        nc.sync.dma_start(out=wg[:, pa:pb, :],
                                          in_=w1_in[:, k0 + pa:k0 + pb, :])
                        nc.sync.dma_start(out=xg[:, pa:pb, :nt],
                                          in_=x_in[c, :, k0 + pa:k0 + pb, :nt])
                    kparts = [(xg, kk, wg, kk) for kk in range(KG)]
                    if c == 0 and g == 4:
                        # Resident weights: first needed by ReLU/L2 at the
                        # end of chunk 0; load late so they don't compete
                        # with the stream while the pipe is still filling.
                        nc.sync.dma_start(out=w2_sb, in_=w2_in[:, :, :])
                        nc.sync.dma_start(out=wh_sb, in_=wh_in[:, :, :])
                        nc.sync.dma_start(out=b1_sb, in_=b1_in[:, :])
                        nc.sync.dma_start(out=b2_sb, in_=b2_in[:, :])
                        nc.sync.dma_start(out=bh_sb, in_=bh_in[:, :])
                    if g < NG - 1:
                        for kk in range(KG):
                            k = k0 + kk
                            xt, xi, wt, wi = kparts[kk]
                            for m in range(MO):
                                nc.tensor.matmul(
                                    ps1[m][:, :nt],
                                    lhsT=wt[:, wi, m * 128:(m + 1) * 128],
                                    rhs=xt[:, xi, :nt],
                                    start=(k == 0),
                                    stop=False,
                                )
                    else:
                        # Last k-group m-major: each m-tile finishes 14 MMs
                        # apart, so its ReLU (and PSUM bank free) pipelines
                        # behind the PE instead of serializing at the end.
                        for m in range(MO):
                            for kk in range(KG):
                                xt, xi, wt, wi = kparts[kk]
                                nc.tensor.matmul(
                                    ps1[m][:, :nt],
                                    lhsT=wt[:, wi, m * 128:(m + 1) * 128],
                                    rhs=xt[:, xi, :nt],
                                    start=False,
                                    stop=(kk == KG - 1),
                                )
                            nc.scalar.activation(
                                h1[:, m, :nt], ps1[m][:, :nt], Relu,
                                bias=b1_sb[:, m:m + 1], scale=1.0,
                            )

                # L2 m2-outer / k2-inner: h1 is resident, so only ~2 PSUM
                # banks stay live and banks free early for the next chunk.
                h2 = h2pool.tile([128, MO, NT], dt.bfloat16, tag="h2",
                                 name=f"h2_{c}")
                for m2 in range(MO):
                    ps2 = pspool.tile([128, NT], dt.float32, tag="mm",
                                      name=f"ps2_{c}_{m2}")
                    for k2 in range(MO):
                        nc.tensor.matmul(
                            ps2[:, :nt],
                            lhsT=w2_sb[:, k2, m2 * 128:(m2 + 1) * 128],
                            rhs=h1[:, k2, :nt],
                            start=(k2 == 0),
                            stop=(k2 == MO - 1),
                        )
                    nc.scalar.activation(
                        h2[:, m2, :nt], ps2[:, :nt], Relu,
                        bias=b2_sb[:, m2:m2 + 1], scale=1.0,
                    )

                psh = pspool.tile([NH, NT], dt.float32, tag="mm",
                                  name=f"psh_{c}")
                for k2 in range(MO):
                    nc.tensor.matmul(
                        psh[:, :nt],
                        lhsT=wh_sb[:, k2, :],
                        rhs=h2[:, k2, :nt],
                        start=(k2 == 0),
                        stop=(k2 == MO - 1),
                    )
                nc.vector.tensor_scalar_add(
                    out=out_sb[:, n0:n0 + nt], in0=psh[:, :nt], scalar1=bh_sb
                )
                nc.sync.dma_start(out=out_t[:, n0:n0 + nt],
                                  in_=out_sb[:, n0:n0 + nt])

    return nc


def _prep_core_x(x_shard_f32):
    """[2500, 12544] f32 -> [CH, 128, KT1, NT] bf16 with
    out[c, p, ko, n] = x[c*NT + n, ko*128 + p] (rows padded with zeros)."""
    xp = np.zeros((ROWS_PAD, D_IN), dtype=BF16)
    xp[:ROWS] = x_shard_f32.astype(BF16)
    v = xp.reshape(CH, NT, KT1, 128)
    return np.ascontiguousarray(np.transpose(v, (0, 3, 2, 1)))


def kernel(x, W1, b1, W2, b2, Wc, bc, Wr, br):
    from concourse.bass_utils import run_bass_kernel_spmd

    x = np.asarray(x, dtype=np.float32)
    W1 = np.asarray(W1, dtype=np.float32)
    W2 = np.asarray(W2, dtype=np.float32)
    Wc = np.asarray(Wc, dtype=np.float32)
    Wr = np.asarray(Wr, dtype=np.float32)
    b1 = np.asarray(b1, dtype=np.float32)
    b2 = np.asarray(b2, dtype=np.float32)
    bc = np.asarray(bc, dtype=np.float32)
    br = np.asarray(br, dtype=np.float32)

    # Weight layouts: [p, ktile, free] with contraction index = ktile*128 + p.
    w1_dev = np.ascontiguousarray(
        W1.astype(BF16).reshape(KT1, 128, H).transpose(1, 0, 2)
    )
    w2_dev = np.ascontiguousarray(
        W2.astype(BF16).reshape(MO, 128, H).transpose(1, 0, 2)
    )
    wh = np.concatenate([Wc, Wr], axis=1)  # [H, 16]
    wh_dev = np.ascontiguousarray(
        wh.astype(BF16).reshape(MO, 128, NH).transpose(1, 0, 2)
    )
    b1_dev = np.ascontiguousarray(b1.reshape(MO, 128).T)
    b2_dev = np.ascontiguousarray(b2.reshape(MO, 128).T)
    bh_dev = np.ascontiguousarray(
        np.concatenate([bc, br]).reshape(NH, 1).astype(np.float32)
    )

    in_maps = []
    for c in range(N_CORES):
        x_dev = _prep_core_x(x[c * ROWS:(c + 1) * ROWS])
        in_maps.append({
            "x_in": x_dev,
            "w1_in": w1_dev,
            "w2_in": w2_dev,
            "wh_in": wh_dev,
            "b1_in": b1_dev,
            "b2_in": b2_dev,
            "bh_in": bh_dev,
        })

    if "nc" not in _CACHE:
        _CACHE["nc"] = _build_nc()
    nc = _CACHE["nc"]

    res = run_bass_kernel_spmd(nc, in_maps, core_ids=list(range(N_CORES)))
    kernel.last_results = res

    outs = []
    for c in range(N_CORES):
        o = res.results[c]["out_t"]          # [16, 2500] f32
        outs.append(o.T)                     # [2500, 16]
    full = np.concatenate(outs, axis=0)      # [20000, 16]
    class_logits = np.ascontiguousarray(full[:, :4])
    box_pred = np.ascontiguousarray(full[:, 4:])
    return class_logits, box_pred
